# revision 33
# baseline (speedup 1.0000x reference)
"""FOFE encoding kernel for Trainium2 (8 NeuronCores, data-parallel over sentences).

Problem: chars [512, 256, 24] (0 = pad), alpha scalar. Per word, z[v] =
sum_k mask_k * alpha^(#non-pad chars after k) * onehot(chars_k)[v], V=256.

Design (per core, 64 sentences = 16384 words laid out [128 partitions x 128
words]):
  1. Weights via a single multiplicative suffix-scan (tensor_tensor_scan with
     per-word reset slots), exact in f32.
  2. Duplicate-char handling on DVE: for shifts s=1..23 accumulate
     priorW_k = sum of weights of earlier same-value chars. Each slot k gets
     data_k = w_k + priorW_k; the LAST occurrence of a value carries the full
     run sum.
  3. GPSIMD local_scatter builds the per-word 256-bin histogram (fp16, 4
     words per call into a 1024-bin dst with per-word 256 offsets). Duplicate
     indices resolve last-write-wins in slot order (HW-probed), so the last
     occurrence's full sum survives. Pads (char 0) carry weight 0 into bin 0.
  4. ScalarE upcasts fp16 -> f32, DMA writes the output.
"""

import sys

if "/opt/trn_rl_repo" not in sys.path:
    sys.path.insert(0, "/opt/trn_rl_repo")

import numpy as np

import concourse.bacc as bacc
import concourse.tile as tile
from concourse import mybir
from concourse.bass_utils import run_bass_kernel_spmd

B, S, W, V = 512, 256, 24, 256
N_CORES = 8
P = 128                      # SBUF partitions
SENT_PER_CORE = B // N_CORES             # 64
WORDS_PER_CORE = SENT_PER_CORE * S       # 16384
WPP = WORDS_PER_CORE // P                # 128 words per partition
GW = 4                       # words per scatter group (GW*V = 1024 dst bins)
SLOTS = W                    # scan slots per word (slot 23 doubles as reset:
                             # its output 0*state+1 = 1 IS char 23's weight)

f32 = mybir.dt.float32
f16 = mybir.dt.float16
bf16 = mybir.dt.bfloat16
i32 = mybir.dt.int32
i16 = mybir.dt.int16
u8 = mybir.dt.uint8
Alu = mybir.AluOpType
Act = mybir.ActivationFunctionType


def build_nc(wpp=WPP, chunks=(40, 48, 28, 12), safe_dedup=False):
    """Build the per-core Bass program.

    chunks: words-per-partition per pipeline chunk (decreasing sizes keep the
    final scatter/store tail short while amortizing DVE op overhead early).

    safe_dedup=True emits explicit last-occurrence masking (no duplicate
    indices ever reach local_scatter) so the program can run under CoreSim,
    which rejects duplicates. The HW path relies on last-write-wins instead.
    """
    assert sum(chunks) == wpp
    cw0 = max(chunks)
    CF0 = cw0 * W
    SF0 = cw0 * SLOTS
    OFFW = ((cw0 + GW - 1) // GW) * GW          # off pattern length in words

    nc = bacc.Bacc("TRN2", target_bir_lowering=False, debug=False,
                   num_devices=N_CORES)
    chars_d = nc.dram_tensor("chars", [P, wpp * W], i32, kind="ExternalInput")
    am1_d = nc.dram_tensor("alpha_m1", [P, 1], f32, kind="ExternalInput")
    z_d = nc.dram_tensor("z", [P, wpp * V], f32, kind="ExternalOutput")

    from contextlib import ExitStack

    with tile.TileContext(nc) as tc, ExitStack() as ctx:
        const = ctx.enter_context(tc.tile_pool(name="const", bufs=1))
        work = ctx.enter_context(tc.tile_pool(name="work", bufs=3))
        scat = ctx.enter_context(tc.tile_pool(name="scat", bufs=12))

        # ---- constants ----
        t_am1 = const.tile([P, 1], f32)
        nc.sync.dma_start(t_am1[:], am1_d[:])

        # off[w*W+k] = (w % GW) * V
        t_off_i = const.tile([P, OFFW * W], i16)
        nc.gpsimd.iota(t_off_i[:], [[0, OFFW // GW], [V, GW], [0, W]],
                       channel_multiplier=0)
        t_off = const.tile([P, OFFW * W], bf16)  # multiples of 256: exact bf16
        nc.scalar.copy(t_off[:], t_off_i[:])

        # scan data1: 1.0 at reset slot (SLOTS-1), else 0
        t_d1 = const.tile([P, SF0], f32)
        nc.gpsimd.memset(t_d1[:], 0.0)
        d1_3 = t_d1[:].rearrange("p (w s) -> p w s", s=SLOTS)
        nc.gpsimd.memset(d1_3[:, :, SLOTS - 1 : SLOTS], 1.0)

        if safe_dedup:
            t_neg1 = const.tile([P, CF0], f32)
            nc.gpsimd.memset(t_neg1[:], -1.0)

        word_bases = [sum(chunks[:i]) for i in range(len(chunks))]

        def front(ch):
            """Load + weights for one chunk: everything up to (c, c2, w)."""
            cw = chunks[ch]
            CF = cw * W
            SF = cw * SLOTS
            st = {"cw": cw, "CF": CF}
            t_ci = work.tile([P, CF], i32, tag="ci")
            nc.sync.dma_start(
                t_ci[:],
                chars_d[:, word_bases[ch] * W : word_bases[ch] * W + CF])

            t_mask = work.tile([P, CF], bf16, tag="mask")
            if ch == 0:
                # chunk 0: DVE is idle at the head; skip the serial ScalarE hop
                nc.vector.tensor_scalar(t_mask[:], t_ci[:], 1.0, None, Alu.min)
            else:
                nc.scalar.activation(t_mask[:], t_ci[:], Act.Sign)

            # per word slots [0..W-1] ~ chars, slot W = reset. The scan runs
            # over the buffer REVERSED so each word segment starts at its
            # reset slot: state = (data0 * state) + data1.
            t_a = work.tile([P, SF], f32, tag="a")
            a_3 = t_a[:].rearrange("p (w s) -> p w s", s=SLOTS)
            m_3 = t_mask[:].rearrange("p (w c) -> p w c", c=W)
            nc.vector.memset(a_3[:, :, W - 1 : W], 0.0)
            nc.scalar.activation(a_3[:, :, 0 : W - 1], m_3[:, :, 1:W],
                                 Act.Copy, bias=1.0, scale=t_am1[:])

            t_cb = work.tile([P, CF], bf16, tag="cb")
            if ch == 0:
                # head: DVE idle, and the first eq op gates on cb
                nc.vector.tensor_copy(t_cb[:], t_ci[:])
            else:
                nc.scalar.copy(t_cb[:], t_ci[:])
            # chars shifted by one slot: for odd shifts s the eq/tmp operands
            # read c2 at even element offsets, keeping the DVE 2x bf16 mode
            # (which needs 4-byte alignment).
            t_c2 = work.tile([P, CF], bf16, tag="c2")
            nc.vector.tensor_copy(t_c2[:][:, 0 : CF - 1], t_ci[:][:, 1:CF])

            t_st = work.tile([P, SF], f32, tag="st")
            nc.vector.tensor_tensor_scan(
                t_st[:][:, ::-1], t_a[:][:, ::-1], t_d1[:][:, 0:SF][:, ::-1],
                0.0, Alu.mult, Alu.add)

            # w in bf16: for alpha = 0.5 every weight is a power of two and
            # bf16 is exact; products eq*w below stay exact.
            st_3 = t_st[:].rearrange("p (w s) -> p w s", s=SLOTS)
            t_w = work.tile([P, CF], bf16, tag="w")
            w_3 = t_w[:].rearrange("p (w c) -> p w c", c=W)
            nc.vector.tensor_tensor(w_3, m_3, st_3[:, :, 0:W], Alu.mult)
            st.update(cb=t_cb, c2=t_c2, w=t_w, w3=w_3)
            return st

        def dedup(st):
            """priorW accumulation + scatter data/idx for one chunk."""
            CF = st["CF"]
            w_3 = st["w3"]
            t_pw = work.tile([P, CF], f16, tag="pw")
            # pw[:, :, 0] has no prior-dup contributions; every other column is
            # initialized by the s=1 copy below, so only column 0 needs zeroing.
            t_eq = work.tile([P, CF], bf16, tag="eq")
            t_tmp = work.tile([P, CF], bf16, tag="tmp")
            c_3 = st["cb"][:].rearrange("p (w c) -> p w c", c=W)
            c2_3 = st["c2"][:].rearrange("p (w c) -> p w c", c=W)
            eq_3 = t_eq[:].rearrange("p (w c) -> p w c", c=W)
            tm_3 = t_tmp[:].rearrange("p (w c) -> p w c", c=W)
            pw_3 = t_pw[:].rearrange("p (w c) -> p w c", c=W)
            nc.vector.memset(pw_3[:, :, 0:1], 0.0)
            if safe_dedup:
                t_lb = work.tile([P, CF], bf16, tag="lb")
                nc.gpsimd.memset(t_lb[:], 0.0)
                lb_3 = t_lb[:].rearrange("p (w c) -> p w c", c=W)
            for s in range(1, W):
                n = W - s
                if s % 2 == 0:
                    shifted = c_3[:, :, s : s + n]
                else:
                    shifted = c2_3[:, :, s - 1 : s - 1 + n]
                nc.vector.tensor_tensor(eq_3[:, :, :n], c_3[:, :, :n],
                                        shifted, Alu.is_equal)
                nc.vector.tensor_tensor(tm_3[:, :, :n], eq_3[:, :, :n],
                                        w_3[:, :, :n], Alu.mult)
                if s == 1:
                    nc.vector.tensor_copy(pw_3[:, :, 1:], tm_3[:, :, :n])
                else:
                    nc.vector.tensor_tensor(pw_3[:, :, s:], pw_3[:, :, s:],
                                            tm_3[:, :, :n], Alu.add)
                if safe_dedup:
                    nc.vector.tensor_tensor(lb_3[:, :, :n], lb_3[:, :, :n],
                                            eq_3[:, :, :n], Alu.max)

            t_data = work.tile([P, CF], f16, tag="data")
            nc.vector.tensor_tensor(t_data[:], st["w"][:], t_pw[:], Alu.add)
            t_idx = work.tile([P, CF], i16, tag="idx")
            if safe_dedup:
                t_L = work.tile([P, CF], u8, tag="L")
                nc.vector.tensor_scalar(t_L[:], t_lb[:], 0.0, None,
                                        Alu.is_equal)
                t_t1 = work.tile([P, CF], f32, tag="t1")
                nc.vector.tensor_tensor(t_t1[:], st["cb"][:],
                                        t_off[:][:, 0:CF], Alu.add)
                nc.vector.select(t_idx[:], t_L[:], t_t1[:],
                                 t_neg1[:][:, 0:CF])
            else:
                nc.vector.tensor_tensor(t_idx[:], st["cb"][:],
                                        t_off[:][:, 0:CF], Alu.add)
            st.update(data=t_data, idx=t_idx)

        def scatter(ch, st):
            zb = word_bases[ch] * V
            last = ch == len(chunks) - 1
            for g0 in range(0, st["cw"], GW):
                r = min(GW, st["cw"] - g0)       # words in this group
                t_dst = scat.tile([P, GW * V], f16, tag="dst")
                nc.gpsimd.local_scatter(
                    t_dst[:][:, 0 : r * V],
                    st["data"][:][:, g0 * W : (g0 + r) * W],
                    st["idx"][:][:, g0 * W : (g0 + r) * W],
                    P, r * V, r * W)
                t_z = scat.tile([P, GW * V], f32, tag="z")
                if last:
                    # tail: DVE is done with compute; its f16->f32 copy is
                    # ~2x faster than ScalarE and shortens the drain chain
                    nc.vector.tensor_copy(t_z[:][:, 0 : r * V],
                                          t_dst[:][:, 0 : r * V])
                else:
                    nc.scalar.copy(t_z[:][:, 0 : r * V], t_dst[:][:, 0 : r * V])
                nc.sync.dma_start(
                    z_d[:, zb + g0 * V : zb + (g0 + r) * V],
                    t_z[:][:, 0 : r * V])

        # software-pipelined emission with two chunks of lookahead (work pool
        # bufs=3 keeps three chunk states live): later chunks' loads/casts are
        # emitted ahead of this chunk's dedup/scatter so ScalarE prioritizes
        # them over upcasts and DVE never waits on a cold chunk.
        states = [None] * len(chunks)
        states[0] = front(0)
        if len(chunks) > 1:
            states[1] = front(1)
        for ch in range(len(chunks)):
            if ch + 2 < len(chunks):
                states[ch + 2] = front(ch + 2)
            dedup(states[ch])
            scatter(ch, states[ch])
            states[ch] = None

    nc.compile()
    return nc


_NC_CACHE = {}


def _get_nc():
    if "nc" not in _NC_CACHE:
        _NC_CACHE["nc"] = build_nc()
    return _NC_CACHE["nc"]


def _marshal(chars_core: np.ndarray, alpha: float):
    # chars_core [SENT_PER_CORE, S, W] -> [P, wpp*W] partition-major
    c = np.ascontiguousarray(
        chars_core.reshape(WORDS_PER_CORE, W).reshape(P, WPP * W)
    ).astype(np.int32)
    am1 = np.full((P, 1), np.float32(alpha) - 1.0, np.float32)
    return {"chars": c, "alpha_m1": am1}


def kernel(chars, lengths, forgetting_factor):
    chars = np.asarray(chars)
    lengths_in = lengths
    alpha = float(np.asarray(forgetting_factor))
    nc = _get_nc()
    in_maps = [
        _marshal(chars[c * SENT_PER_CORE : (c + 1) * SENT_PER_CORE], alpha)
        for c in range(N_CORES)
    ]
    res = run_bass_kernel_spmd(nc, in_maps, list(range(N_CORES)))
    z = np.empty((B, S, V), np.float32)
    for c in range(N_CORES):
        zc = res.results[c]["z"].reshape(WORDS_PER_CORE, V)
        z[c * SENT_PER_CORE : (c + 1) * SENT_PER_CORE] = zc.reshape(
            SENT_PER_CORE, S, V)
    return z, np.asarray(lengths_in)


# revision 35
# speedup vs baseline: 1.0031x; 1.0031x over previous
"""FOFE encoding kernel for Trainium2 (8 NeuronCores, data-parallel over sentences).

Problem: chars [512, 256, 24] (0 = pad), alpha scalar. Per word, z[v] =
sum_k mask_k * alpha^(#non-pad chars after k) * onehot(chars_k)[v], V=256.

Design (per core, 64 sentences = 16384 words laid out [128 partitions x 128
words]):
  1. Weights via a single multiplicative suffix-scan (tensor_tensor_scan with
     per-word reset slots), exact in f32.
  2. Duplicate-char handling on DVE: for shifts s=1..23 accumulate
     priorW_k = sum of weights of earlier same-value chars. Each slot k gets
     data_k = w_k + priorW_k; the LAST occurrence of a value carries the full
     run sum.
  3. GPSIMD local_scatter builds the per-word 256-bin histogram (fp16, 4
     words per call into a 1024-bin dst with per-word 256 offsets). Duplicate
     indices resolve last-write-wins in slot order (HW-probed), so the last
     occurrence's full sum survives. Pads (char 0) carry weight 0 into bin 0.
  4. ScalarE upcasts fp16 -> f32, DMA writes the output.
"""

import sys

if "/opt/trn_rl_repo" not in sys.path:
    sys.path.insert(0, "/opt/trn_rl_repo")

import numpy as np

import concourse.bacc as bacc
import concourse.tile as tile
from concourse import mybir
from concourse.bass_utils import run_bass_kernel_spmd

B, S, W, V = 512, 256, 24, 256
N_CORES = 8
P = 128                      # SBUF partitions
SENT_PER_CORE = B // N_CORES             # 64
WORDS_PER_CORE = SENT_PER_CORE * S       # 16384
WPP = WORDS_PER_CORE // P                # 128 words per partition
GW = 4                       # words per scatter group (GW*V = 1024 dst bins)
SLOTS = W                    # scan slots per word (slot 23 doubles as reset:
                             # its output 0*state+1 = 1 IS char 23's weight)

f32 = mybir.dt.float32
f16 = mybir.dt.float16
bf16 = mybir.dt.bfloat16
i32 = mybir.dt.int32
i16 = mybir.dt.int16
u8 = mybir.dt.uint8
Alu = mybir.AluOpType
Act = mybir.ActivationFunctionType


def build_nc(wpp=WPP, chunks=(40, 44, 32, 12), safe_dedup=False):
    """Build the per-core Bass program.

    chunks: words-per-partition per pipeline chunk (decreasing sizes keep the
    final scatter/store tail short while amortizing DVE op overhead early).

    safe_dedup=True emits explicit last-occurrence masking (no duplicate
    indices ever reach local_scatter) so the program can run under CoreSim,
    which rejects duplicates. The HW path relies on last-write-wins instead.
    """
    assert sum(chunks) == wpp
    cw0 = max(chunks)
    CF0 = cw0 * W
    SF0 = cw0 * SLOTS
    OFFW = ((cw0 + GW - 1) // GW) * GW          # off pattern length in words

    nc = bacc.Bacc("TRN2", target_bir_lowering=False, debug=False,
                   num_devices=N_CORES)
    chars_d = nc.dram_tensor("chars", [P, wpp * W], i32, kind="ExternalInput")
    am1_d = nc.dram_tensor("alpha_m1", [P, 1], f32, kind="ExternalInput")
    z_d = nc.dram_tensor("z", [P, wpp * V], f32, kind="ExternalOutput")

    from contextlib import ExitStack

    with tile.TileContext(nc) as tc, ExitStack() as ctx:
        const = ctx.enter_context(tc.tile_pool(name="const", bufs=1))
        work = ctx.enter_context(tc.tile_pool(name="work", bufs=3))
        scat = ctx.enter_context(tc.tile_pool(name="scat", bufs=10))

        # ---- constants ----
        t_am1 = const.tile([P, 1], f32)
        nc.sync.dma_start(t_am1[:], am1_d[:])

        # off[w*W+k] = (w % GW) * V
        t_off_i = const.tile([P, OFFW * W], i16)
        nc.gpsimd.iota(t_off_i[:], [[0, OFFW // GW], [V, GW], [0, W]],
                       channel_multiplier=0)
        t_off = const.tile([P, OFFW * W], bf16)  # multiples of 256: exact bf16
        nc.scalar.copy(t_off[:], t_off_i[:])

        # scan data1: 1.0 at reset slot (SLOTS-1), else 0
        t_d1 = const.tile([P, SF0], f32)
        nc.gpsimd.memset(t_d1[:], 0.0)
        d1_3 = t_d1[:].rearrange("p (w s) -> p w s", s=SLOTS)
        nc.gpsimd.memset(d1_3[:, :, SLOTS - 1 : SLOTS], 1.0)

        if safe_dedup:
            t_neg1 = const.tile([P, CF0], f32)
            nc.gpsimd.memset(t_neg1[:], -1.0)

        word_bases = [sum(chunks[:i]) for i in range(len(chunks))]

        def front(ch):
            """Load + weights for one chunk: everything up to (c, c2, w)."""
            cw = chunks[ch]
            CF = cw * W
            SF = cw * SLOTS
            st = {"cw": cw, "CF": CF}
            t_ci = work.tile([P, CF], i32, tag="ci")
            nc.sync.dma_start(
                t_ci[:],
                chars_d[:, word_bases[ch] * W : word_bases[ch] * W + CF])

            t_mask = work.tile([P, CF], bf16, tag="mask")
            if ch == 0:
                # chunk 0: DVE is idle at the head; skip the serial ScalarE hop
                nc.vector.tensor_scalar(t_mask[:], t_ci[:], 1.0, None, Alu.min)
            else:
                nc.scalar.activation(t_mask[:], t_ci[:], Act.Sign)

            # per word slots [0..W-1] ~ chars, slot W = reset. The scan runs
            # over the buffer REVERSED so each word segment starts at its
            # reset slot: state = (data0 * state) + data1.
            t_a = work.tile([P, SF], f32, tag="a")
            a_3 = t_a[:].rearrange("p (w s) -> p w s", s=SLOTS)
            m_3 = t_mask[:].rearrange("p (w c) -> p w c", c=W)
            nc.vector.memset(a_3[:, :, W - 1 : W], 0.0)
            nc.scalar.activation(a_3[:, :, 0 : W - 1], m_3[:, :, 1:W],
                                 Act.Copy, bias=1.0, scale=t_am1[:])

            t_cb = work.tile([P, CF], bf16, tag="cb")
            nc.scalar.copy(t_cb[:], t_ci[:])
            # chars shifted by one slot: for odd shifts s the eq/tmp operands
            # read c2 at even element offsets, keeping the DVE 2x bf16 mode
            # (which needs 4-byte alignment).
            t_c2 = work.tile([P, CF], bf16, tag="c2")
            nc.vector.tensor_copy(t_c2[:][:, 0 : CF - 1], t_ci[:][:, 1:CF])

            t_st = work.tile([P, SF], f32, tag="st")
            nc.vector.tensor_tensor_scan(
                t_st[:][:, ::-1], t_a[:][:, ::-1], t_d1[:][:, 0:SF][:, ::-1],
                0.0, Alu.mult, Alu.add)

            # w in bf16: for alpha = 0.5 every weight is a power of two and
            # bf16 is exact; products eq*w below stay exact.
            st_3 = t_st[:].rearrange("p (w s) -> p w s", s=SLOTS)
            t_w = work.tile([P, CF], bf16, tag="w")
            w_3 = t_w[:].rearrange("p (w c) -> p w c", c=W)
            nc.vector.tensor_tensor(w_3, m_3, st_3[:, :, 0:W], Alu.mult)
            st.update(cb=t_cb, c2=t_c2, w=t_w, w3=w_3)
            return st

        def dedup(st):
            """priorW accumulation + scatter data/idx for one chunk."""
            CF = st["CF"]
            w_3 = st["w3"]
            t_pw = work.tile([P, CF], f16, tag="pw")
            # pw[:, :, 0] has no prior-dup contributions; every other column is
            # initialized by the s=1 copy below, so only column 0 needs zeroing.
            t_eq = work.tile([P, CF], bf16, tag="eq")
            t_tmp = work.tile([P, CF], bf16, tag="tmp")
            c_3 = st["cb"][:].rearrange("p (w c) -> p w c", c=W)
            c2_3 = st["c2"][:].rearrange("p (w c) -> p w c", c=W)
            eq_3 = t_eq[:].rearrange("p (w c) -> p w c", c=W)
            tm_3 = t_tmp[:].rearrange("p (w c) -> p w c", c=W)
            pw_3 = t_pw[:].rearrange("p (w c) -> p w c", c=W)
            nc.vector.memset(pw_3[:, :, 0:1], 0.0)
            if safe_dedup:
                t_lb = work.tile([P, CF], bf16, tag="lb")
                nc.gpsimd.memset(t_lb[:], 0.0)
                lb_3 = t_lb[:].rearrange("p (w c) -> p w c", c=W)
            for s in range(1, W):
                n = W - s
                if s % 2 == 0:
                    shifted = c_3[:, :, s : s + n]
                else:
                    shifted = c2_3[:, :, s - 1 : s - 1 + n]
                nc.vector.tensor_tensor(eq_3[:, :, :n], c_3[:, :, :n],
                                        shifted, Alu.is_equal)
                nc.vector.tensor_tensor(tm_3[:, :, :n], eq_3[:, :, :n],
                                        w_3[:, :, :n], Alu.mult)
                if s == 1:
                    nc.vector.tensor_copy(pw_3[:, :, 1:], tm_3[:, :, :n])
                else:
                    nc.vector.tensor_tensor(pw_3[:, :, s:], pw_3[:, :, s:],
                                            tm_3[:, :, :n], Alu.add)
                if safe_dedup:
                    nc.vector.tensor_tensor(lb_3[:, :, :n], lb_3[:, :, :n],
                                            eq_3[:, :, :n], Alu.max)

            t_data = work.tile([P, CF], f16, tag="data")
            nc.vector.tensor_tensor(t_data[:], st["w"][:], t_pw[:], Alu.add)
            t_idx = work.tile([P, CF], i16, tag="idx")
            if safe_dedup:
                t_L = work.tile([P, CF], u8, tag="L")
                nc.vector.tensor_scalar(t_L[:], t_lb[:], 0.0, None,
                                        Alu.is_equal)
                t_t1 = work.tile([P, CF], f32, tag="t1")
                nc.vector.tensor_tensor(t_t1[:], st["cb"][:],
                                        t_off[:][:, 0:CF], Alu.add)
                nc.vector.select(t_idx[:], t_L[:], t_t1[:],
                                 t_neg1[:][:, 0:CF])
            else:
                nc.vector.tensor_tensor(t_idx[:], st["cb"][:],
                                        t_off[:][:, 0:CF], Alu.add)
            st.update(data=t_data, idx=t_idx)

        def scatter(ch, st):
            zb = word_bases[ch] * V
            last = ch == len(chunks) - 1
            for g0 in range(0, st["cw"], GW):
                r = min(GW, st["cw"] - g0)       # words in this group
                t_dst = scat.tile([P, GW * V], f16, tag="dst")
                nc.gpsimd.local_scatter(
                    t_dst[:][:, 0 : r * V],
                    st["data"][:][:, g0 * W : (g0 + r) * W],
                    st["idx"][:][:, g0 * W : (g0 + r) * W],
                    P, r * V, r * W)
                t_z = scat.tile([P, GW * V], f32, tag="z")
                if last:
                    # tail: DVE is done with compute; its f16->f32 copy is
                    # ~2x faster than ScalarE and shortens the drain chain
                    nc.vector.tensor_copy(t_z[:][:, 0 : r * V],
                                          t_dst[:][:, 0 : r * V])
                else:
                    nc.scalar.copy(t_z[:][:, 0 : r * V], t_dst[:][:, 0 : r * V])
                nc.sync.dma_start(
                    z_d[:, zb + g0 * V : zb + (g0 + r) * V],
                    t_z[:][:, 0 : r * V])

        # software-pipelined emission with two chunks of lookahead (work pool
        # bufs=3 keeps three chunk states live): later chunks' loads/casts are
        # emitted ahead of this chunk's dedup/scatter so ScalarE prioritizes
        # them over upcasts and DVE never waits on a cold chunk.
        states = [None] * len(chunks)
        states[0] = front(0)
        if len(chunks) > 1:
            states[1] = front(1)
        for ch in range(len(chunks)):
            if ch + 2 < len(chunks):
                states[ch + 2] = front(ch + 2)
            dedup(states[ch])
            scatter(ch, states[ch])
            states[ch] = None

    nc.compile()
    return nc


_NC_CACHE = {}


def _get_nc():
    if "nc" not in _NC_CACHE:
        _NC_CACHE["nc"] = build_nc()
    return _NC_CACHE["nc"]


def _marshal(chars_core: np.ndarray, alpha: float):
    # chars_core [SENT_PER_CORE, S, W] -> [P, wpp*W] partition-major
    c = np.ascontiguousarray(
        chars_core.reshape(WORDS_PER_CORE, W).reshape(P, WPP * W)
    ).astype(np.int32)
    am1 = np.full((P, 1), np.float32(alpha) - 1.0, np.float32)
    return {"chars": c, "alpha_m1": am1}


def kernel(chars, lengths, forgetting_factor):
    chars = np.asarray(chars)
    lengths_in = lengths
    alpha = float(np.asarray(forgetting_factor))
    nc = _get_nc()
    in_maps = [
        _marshal(chars[c * SENT_PER_CORE : (c + 1) * SENT_PER_CORE], alpha)
        for c in range(N_CORES)
    ]
    res = run_bass_kernel_spmd(nc, in_maps, list(range(N_CORES)))
    z = np.empty((B, S, V), np.float32)
    for c in range(N_CORES):
        zc = res.results[c]["z"].reshape(WORDS_PER_CORE, V)
        z[c * SENT_PER_CORE : (c + 1) * SENT_PER_CORE] = zc.reshape(
            SENT_PER_CORE, S, V)
    return z, np.asarray(lengths_in)


# revision 37
# speedup vs baseline: 1.0238x; 1.0206x over previous
"""FOFE encoding kernel for Trainium2 (8 NeuronCores, data-parallel over sentences).

Problem: chars [512, 256, 24] (0 = pad), alpha scalar. Per word, z[v] =
sum_k mask_k * alpha^(#non-pad chars after k) * onehot(chars_k)[v], V=256.

Design (per core, 64 sentences = 16384 words laid out [128 partitions x 128
words]):
  1. Weights via a single multiplicative suffix-scan (tensor_tensor_scan with
     per-word reset slots), exact in f32.
  2. Duplicate-char handling on DVE: for shifts s=1..23 accumulate
     priorW_k = sum of weights of earlier same-value chars. Each slot k gets
     data_k = w_k + priorW_k; the LAST occurrence of a value carries the full
     run sum.
  3. GPSIMD local_scatter builds the per-word 256-bin histogram (fp16, 4
     words per call into a 1024-bin dst with per-word 256 offsets). Duplicate
     indices resolve last-write-wins in slot order (HW-probed), so the last
     occurrence's full sum survives. Pads (char 0) carry weight 0 into bin 0.
  4. ScalarE upcasts fp16 -> f32, DMA writes the output.
"""

import sys

if "/opt/trn_rl_repo" not in sys.path:
    sys.path.insert(0, "/opt/trn_rl_repo")

import numpy as np

import concourse.bacc as bacc
import concourse.tile as tile
from concourse import mybir
from concourse.bass_utils import run_bass_kernel_spmd

B, S, W, V = 512, 256, 24, 256
N_CORES = 8
P = 128                      # SBUF partitions
SENT_PER_CORE = B // N_CORES             # 64
WORDS_PER_CORE = SENT_PER_CORE * S       # 16384
WPP = WORDS_PER_CORE // P                # 128 words per partition
GW = 4                       # words per scatter group (GW*V = 1024 dst bins)
SLOTS = W                    # scan slots per word (slot 23 doubles as reset:
                             # its output 0*state+1 = 1 IS char 23's weight)

f32 = mybir.dt.float32
f16 = mybir.dt.float16
bf16 = mybir.dt.bfloat16
i32 = mybir.dt.int32
i16 = mybir.dt.int16
u8 = mybir.dt.uint8
Alu = mybir.AluOpType
Act = mybir.ActivationFunctionType


def build_nc(wpp=WPP, chunks=(40, 48, 28, 12), safe_dedup=False):
    """Build the per-core Bass program.

    chunks: words-per-partition per pipeline chunk (decreasing sizes keep the
    final scatter/store tail short while amortizing DVE op overhead early).

    safe_dedup=True emits explicit last-occurrence masking (no duplicate
    indices ever reach local_scatter) so the program can run under CoreSim,
    which rejects duplicates. The HW path relies on last-write-wins instead.
    """
    assert sum(chunks) == wpp
    cw0 = max(chunks)
    CF0 = cw0 * W
    SF0 = cw0 * SLOTS
    OFFW = ((cw0 + GW - 1) // GW) * GW          # off pattern length in words

    nc = bacc.Bacc("TRN2", target_bir_lowering=False, debug=False,
                   num_devices=N_CORES)
    chars_d = nc.dram_tensor("chars", [P, wpp * W], i32, kind="ExternalInput")
    am1_d = nc.dram_tensor("alpha_m1", [P, 1], f32, kind="ExternalInput")
    z_d = nc.dram_tensor("z", [P, wpp * V], f32, kind="ExternalOutput")

    from contextlib import ExitStack

    with tile.TileContext(nc) as tc, ExitStack() as ctx:
        const = ctx.enter_context(tc.tile_pool(name="const", bufs=1))
        work = ctx.enter_context(tc.tile_pool(name="work", bufs=3))
        scat = ctx.enter_context(tc.tile_pool(name="scat", bufs=11))

        # ---- constants ----
        t_am1 = const.tile([P, 1], f32)
        nc.sync.dma_start(t_am1[:], am1_d[:])

        # off[w*W+k] = (w % GW) * V
        t_off_i = const.tile([P, OFFW * W], i16)
        nc.gpsimd.iota(t_off_i[:], [[0, OFFW // GW], [V, GW], [0, W]],
                       channel_multiplier=0)
        t_off = const.tile([P, OFFW * W], bf16)  # multiples of 256: exact bf16
        nc.scalar.copy(t_off[:], t_off_i[:])

        # scan data1: 1.0 at reset slot (SLOTS-1), else 0
        t_d1 = const.tile([P, SF0], f32)
        nc.gpsimd.memset(t_d1[:], 0.0)
        d1_3 = t_d1[:].rearrange("p (w s) -> p w s", s=SLOTS)
        nc.gpsimd.memset(d1_3[:, :, SLOTS - 1 : SLOTS], 1.0)

        if safe_dedup:
            t_neg1 = const.tile([P, CF0], f32)
            nc.gpsimd.memset(t_neg1[:], -1.0)

        word_bases = [sum(chunks[:i]) for i in range(len(chunks))]

        def front(ch):
            """Load + weights for one chunk: everything up to (c, c2, w)."""
            cw = chunks[ch]
            CF = cw * W
            SF = cw * SLOTS
            st = {"cw": cw, "CF": CF}
            t_ci = work.tile([P, CF], i32, tag="ci")
            nc.sync.dma_start(
                t_ci[:],
                chars_d[:, word_bases[ch] * W : word_bases[ch] * W + CF])

            t_mask = work.tile([P, CF], bf16, tag="mask")
            if ch == 0:
                # chunk 0: DVE is idle at the head; skip the serial ScalarE hop
                nc.vector.tensor_scalar(t_mask[:], t_ci[:], 1.0, None, Alu.min)
            else:
                nc.scalar.activation(t_mask[:], t_ci[:], Act.Sign)

            # per word slots [0..W-1] ~ chars, slot W = reset. The scan runs
            # over the buffer REVERSED so each word segment starts at its
            # reset slot: state = (data0 * state) + data1.
            t_a = work.tile([P, SF], f32, tag="a")
            a_3 = t_a[:].rearrange("p (w s) -> p w s", s=SLOTS)
            m_3 = t_mask[:].rearrange("p (w c) -> p w c", c=W)
            nc.vector.memset(a_3[:, :, W - 1 : W], 0.0)
            nc.scalar.activation(a_3[:, :, 0 : W - 1], m_3[:, :, 1:W],
                                 Act.Copy, bias=1.0, scale=t_am1[:])

            t_cb = work.tile([P, CF], bf16, tag="cb")
            nc.scalar.copy(t_cb[:], t_ci[:])
            # chars shifted by one slot: for odd shifts s the eq/tmp operands
            # read c2 at even element offsets, keeping the DVE 2x bf16 mode
            # (which needs 4-byte alignment).
            t_c2 = work.tile([P, CF], bf16, tag="c2")
            nc.vector.tensor_copy(t_c2[:][:, 0 : CF - 1], t_ci[:][:, 1:CF])

            t_st = work.tile([P, SF], f32, tag="st")
            nc.vector.tensor_tensor_scan(
                t_st[:][:, ::-1], t_a[:][:, ::-1], t_d1[:][:, 0:SF][:, ::-1],
                0.0, Alu.mult, Alu.add)

            # w in bf16: for alpha = 0.5 every weight is a power of two and
            # bf16 is exact; products eq*w below stay exact.
            st_3 = t_st[:].rearrange("p (w s) -> p w s", s=SLOTS)
            t_w = work.tile([P, CF], bf16, tag="w")
            w_3 = t_w[:].rearrange("p (w c) -> p w c", c=W)
            nc.vector.tensor_tensor(w_3, m_3, st_3[:, :, 0:W], Alu.mult)
            st.update(cb=t_cb, c2=t_c2, w=t_w, w3=w_3)
            return st

        def dedup(st):
            """priorW accumulation + scatter data/idx for one chunk."""
            CF = st["CF"]
            w_3 = st["w3"]
            t_pw = work.tile([P, CF], f16, tag="pw")
            # pw[:, :, 0] has no prior-dup contributions; every other column is
            # initialized by the s=1 copy below, so only column 0 needs zeroing.
            t_eq = work.tile([P, CF], bf16, tag="eq")
            t_tmp = work.tile([P, CF], bf16, tag="tmp")
            c_3 = st["cb"][:].rearrange("p (w c) -> p w c", c=W)
            c2_3 = st["c2"][:].rearrange("p (w c) -> p w c", c=W)
            eq_3 = t_eq[:].rearrange("p (w c) -> p w c", c=W)
            tm_3 = t_tmp[:].rearrange("p (w c) -> p w c", c=W)
            pw_3 = t_pw[:].rearrange("p (w c) -> p w c", c=W)
            nc.vector.memset(pw_3[:, :, 0:1], 0.0)
            if safe_dedup:
                t_lb = work.tile([P, CF], bf16, tag="lb")
                nc.gpsimd.memset(t_lb[:], 0.0)
                lb_3 = t_lb[:].rearrange("p (w c) -> p w c", c=W)
            for s in range(1, W):
                n = W - s
                if s % 2 == 0:
                    shifted = c_3[:, :, s : s + n]
                else:
                    shifted = c2_3[:, :, s - 1 : s - 1 + n]
                nc.vector.tensor_tensor(eq_3[:, :, :n], c_3[:, :, :n],
                                        shifted, Alu.is_equal)
                nc.vector.tensor_tensor(tm_3[:, :, :n], eq_3[:, :, :n],
                                        w_3[:, :, :n], Alu.mult)
                if s == 1:
                    nc.vector.tensor_copy(pw_3[:, :, 1:], tm_3[:, :, :n])
                else:
                    nc.vector.tensor_tensor(pw_3[:, :, s:], pw_3[:, :, s:],
                                            tm_3[:, :, :n], Alu.add)
                if safe_dedup:
                    nc.vector.tensor_tensor(lb_3[:, :, :n], lb_3[:, :, :n],
                                            eq_3[:, :, :n], Alu.max)

            t_data = work.tile([P, CF], f16, tag="data")
            nc.vector.tensor_tensor(t_data[:], st["w"][:], t_pw[:], Alu.add)
            t_idx = work.tile([P, CF], i16, tag="idx")
            if safe_dedup:
                t_L = work.tile([P, CF], u8, tag="L")
                nc.vector.tensor_scalar(t_L[:], t_lb[:], 0.0, None,
                                        Alu.is_equal)
                t_t1 = work.tile([P, CF], f32, tag="t1")
                nc.vector.tensor_tensor(t_t1[:], st["cb"][:],
                                        t_off[:][:, 0:CF], Alu.add)
                nc.vector.select(t_idx[:], t_L[:], t_t1[:],
                                 t_neg1[:][:, 0:CF])
            else:
                nc.vector.tensor_tensor(t_idx[:], st["cb"][:],
                                        t_off[:][:, 0:CF], Alu.add)
            st.update(data=t_data, idx=t_idx)

        def scatter(ch, st):
            zb = word_bases[ch] * V
            last = ch == len(chunks) - 1
            for g0 in range(0, st["cw"], GW):
                r = min(GW, st["cw"] - g0)       # words in this group
                t_dst = scat.tile([P, GW * V], f16, tag="dst")
                nc.gpsimd.local_scatter(
                    t_dst[:][:, 0 : r * V],
                    st["data"][:][:, g0 * W : (g0 + r) * W],
                    st["idx"][:][:, g0 * W : (g0 + r) * W],
                    P, r * V, r * W)
                t_z = scat.tile([P, GW * V], f32, tag="z")
                if last:
                    # tail: DVE is done with compute; its f16->f32 copy is
                    # ~2x faster than ScalarE and shortens the drain chain
                    nc.vector.tensor_copy(t_z[:][:, 0 : r * V],
                                          t_dst[:][:, 0 : r * V])
                else:
                    nc.scalar.copy(t_z[:][:, 0 : r * V], t_dst[:][:, 0 : r * V])
                nc.sync.dma_start(
                    z_d[:, zb + g0 * V : zb + (g0 + r) * V],
                    t_z[:][:, 0 : r * V])

        # software-pipelined emission with two chunks of lookahead (work pool
        # bufs=3 keeps three chunk states live): later chunks' loads/casts are
        # emitted ahead of this chunk's dedup/scatter so ScalarE prioritizes
        # them over upcasts and DVE never waits on a cold chunk.
        states = [None] * len(chunks)
        states[0] = front(0)
        if len(chunks) > 1:
            states[1] = front(1)
        for ch in range(len(chunks)):
            if ch + 2 < len(chunks):
                states[ch + 2] = front(ch + 2)
            dedup(states[ch])
            scatter(ch, states[ch])
            states[ch] = None

    nc.compile()
    return nc


_NC_CACHE = {}


def _get_nc():
    if "nc" not in _NC_CACHE:
        _NC_CACHE["nc"] = build_nc()
    return _NC_CACHE["nc"]


def _marshal(chars_core: np.ndarray, alpha: float):
    # chars_core [SENT_PER_CORE, S, W] -> [P, wpp*W] partition-major
    c = np.ascontiguousarray(
        chars_core.reshape(WORDS_PER_CORE, W).reshape(P, WPP * W)
    ).astype(np.int32)
    am1 = np.full((P, 1), np.float32(alpha) - 1.0, np.float32)
    return {"chars": c, "alpha_m1": am1}


def kernel(chars, lengths, forgetting_factor):
    chars = np.asarray(chars)
    lengths_in = lengths
    alpha = float(np.asarray(forgetting_factor))
    nc = _get_nc()
    in_maps = [
        _marshal(chars[c * SENT_PER_CORE : (c + 1) * SENT_PER_CORE], alpha)
        for c in range(N_CORES)
    ]
    res = run_bass_kernel_spmd(nc, in_maps, list(range(N_CORES)))
    z = np.empty((B, S, V), np.float32)
    for c in range(N_CORES):
        zc = res.results[c]["z"].reshape(WORDS_PER_CORE, V)
        z[c * SENT_PER_CORE : (c + 1) * SENT_PER_CORE] = zc.reshape(
            SENT_PER_CORE, S, V)
    return z, np.asarray(lengths_in)
